# revision 7
# baseline (speedup 1.0000x reference)
"""Trainium2 Bass kernel for the Mamba-style selective-scan block
(nn_Block_24962349924931).

Shapes: x [2, 4096, 1024]; D_MODEL=1024, D_INNER=2048, D_STATE=16, K=3.

Sharding: 8 cores = DP2 (batch) x TP4 (d_inner channels, 512/core).
Two device launches with a host-side exchange of u between them:

  Launch A: u = silu(im2col(x) @ W_fused + b_fused)   [per-core channel shard]
            (W_fused = in_w @ conv_w, fused on host), plus B/C projection
            PARTIALS contracted over the core's own channel shard.
  host: gather u shards -> full u per batch; sum B/C partials, add biases,
        pre-broadcast B/C rows to 128 partitions.
  Launch B: delta via dt-proj matmuls + softplus (2 fused Act ops:
            e=exp(z+b), delta=ln(e+1)); per-STATE channel-layout scan:
            dA_n = exp(A_n*delta) built by a square-chain on DVE (even
            powers) + Act exps (odd powers); X_n = (delta*u) o bbrd_n;
            h_n = tensor_tensor_scan; ch_n = h_n o ccrd_n; y = sum_n ch_n
            via identity-matmul PSUM accumulation; out_partial =
            (y + u*Dskip) @ out_w^T streamed PSUM->DRAM as fp32.
  host: sum the 4 TP partials per batch, add out_b, stack batches.
"""
import sys
sys.path.insert(0, "/opt/trn_rl_repo")

import numpy as np
import ml_dtypes

import concourse.bacc as bacc
import concourse.mybir as mybir
from concourse import bass_utils
from concourse.tile import TileContext

F32 = mybir.dt.float32
BF16 = mybir.dt.bfloat16
FP16 = mybir.dt.float16
AL = mybir.AluOpType
AF = mybir.ActivationFunctionType
BF16NP = ml_dtypes.bfloat16

# ---- problem constants ----
B, L, D, E, N, K = 2, 4096, 1024, 2048, 16, 3
NCORES = 8
TPG = 4              # cores per batch (tensor parallel group)
EL = E // TPG        # 512 channels per core
FT = EL // 128       # 4 f-tiles per core
KT_U = E // 128      # 16 k-tiles over full E
KT_X = (D * K) // 128  # 24 k-tiles over im2col contraction
TC = L // 512        # 8 512-chunks (launch A)
CW = 1024            # launch B elementwise chunk width
NCH = L // CW        # 4 chunks
SEG = CW // 512      # 512-wide PSUM segments per chunk

# engine split tunables for stage-2 of launch B
CH_ON_GP = tuple(n % 4 == 3 for n in range(N))   # ch-mult -> GpSimd for these
X_ON_GP = tuple(False for n in range(N))
FP8_DT = False        # dt-proj via fp8e4 DoubleRow matmuls
U8_SCALE = 32.0       # u -> fp8 scale
DTW8_SCALE = 2048.0   # dt_w -> fp8 scale
FP8E4 = mybir.dt.float8e4
E4M3NP = ml_dtypes.float8_e4m3


def _bf16(a):
    return np.ascontiguousarray(np.asarray(a).astype(BF16NP))


# ===================================================================
# Launch A: u = silu(im2col(x) @ W_fused + b_fused);  B/C partials
# ===================================================================
def build_launch_a(loop_reps=0, emit_fp8=False):
    nc = bacc.Bacc("TRN2", target_bir_lowering=False, debug=False)
    xT = nc.dram_tensor("xT", [D, L + 2], BF16, kind="ExternalInput")
    wf = nc.dram_tensor("wf", [KT_X, 128, EL], BF16, kind="ExternalInput")
    bfu = nc.dram_tensor("bfu", [128, FT], F32, kind="ExternalInput")
    bcw = nc.dram_tensor("bcw", [FT, 128, 2 * N], BF16, kind="ExternalInput")
    u_out = nc.dram_tensor("u_out", [EL, L], BF16, kind="ExternalOutput")
    bcp = nc.dram_tensor("bcp", [2 * N, L], F32, kind="ExternalOutput")
    if emit_fp8:
        u8_out = nc.dram_tensor("u8_out", [EL, L], FP8E4,
                                kind="ExternalOutput")

    with TileContext(nc) as tc:
        if loop_reps:
            tc.race_detector_enabled = False
        with (
            tc.tile_pool(name="big", bufs=1) as big,
            tc.tile_pool(name="work", bufs=2) as work,
            tc.tile_pool(name="ps", bufs=6, space="PSUM") as ps,
            tc.tile_pool(name="psbc", bufs=2, space="PSUM") as psbc,
        ):
            xsb = big.tile([128, 8 * (L + 2)], BF16, tag="xsb", name="xsb")
            for j in range(8):
                nc.sync.dma_start(
                    xsb[:, j * (L + 2):(j + 1) * (L + 2)],
                    xT[j * 128:(j + 1) * 128, :])
            wfsb = big.tile([128, KT_X * EL], BF16, tag="wfsb", name="wfsb")
            for kt in range(KT_X):
                nc.sync.dma_start(wfsb[:, kt * EL:(kt + 1) * EL], wf[kt, :, :])
            bfu_t = big.tile([128, FT], F32, tag="bfu", name="bfu")
            nc.sync.dma_start(bfu_t[:, :], bfu[:, :])
            bcw_t = big.tile([128, FT * 2 * N], BF16, tag="bcw", name="bcw")
            for ft in range(FT):
                nc.sync.dma_start(bcw_t[:, ft * 2 * N:(ft + 1) * 2 * N],
                                  bcw[ft, :, :])

            def body(_=None, unroll=None):
                for tcI in range(TC):
                    t0 = tcI * 512
                    uts = []
                    for ft in range(FT):
                        pt = ps.tile([128, 512], F32, tag="acc", name="acc")
                        for kt in range(KT_X):
                            kap, j = divmod(kt, 8)
                            rhs = xsb[:, j * (L + 2) + t0 + kap:
                                      j * (L + 2) + t0 + kap + 512]
                            lhsT = wfsb[:, kt * EL + ft * 128:
                                        kt * EL + (ft + 1) * 128]
                            nc.tensor.matmul(pt[:, :], lhsT, rhs,
                                             start=(kt == 0),
                                             stop=(kt == KT_X - 1))
                        ut = work.tile([128, 512], BF16, tag=f"u{ft}",
                                       name=f"u{ft}")
                        nc.scalar.activation(ut[:, :], pt[:, :], AF.Silu,
                                             bias=bfu_t[:, ft:ft + 1])
                        nc.sync.dma_start(
                            u_out[ft * 128:(ft + 1) * 128, t0:t0 + 512],
                            ut[:, :])
                        if emit_fp8:
                            u8t = work.tile([128, 512], FP8E4, tag=f"u8{ft}",
                                            name=f"u8{ft}")
                            nc.scalar.activation(u8t[:, :], ut[:, :],
                                                 AF.Copy, scale=U8_SCALE)
                            nc.sync.dma_start(
                                u8_out[ft * 128:(ft + 1) * 128, t0:t0 + 512],
                                u8t[:, :])
                        uts.append(ut)
                    pbc = psbc.tile([2 * N, 512], F32, tag="pbc", name="pbc")
                    for ft in range(FT):
                        nc.tensor.matmul(
                            pbc[:, :],
                            bcw_t[:, ft * 2 * N:(ft + 1) * 2 * N],
                            uts[ft][:, :], start=(ft == 0),
                            stop=(ft == FT - 1))
                    bcs = work.tile([2 * N, 512], F32, tag="bcs", name="bcs")
                    nc.scalar.activation(bcs[:, :], pbc[:, :], AF.Copy)
                    nc.sync.dma_start(bcp[:, t0:t0 + 512], bcs[:, :])

            if loop_reps:
                with tc.For_i(0, loop_reps, 1) as _i:
                    body()
            else:
                body()
    nc.compile()
    return nc


# ===================================================================
# Launch B: dt-proj + per-state scan + out-proj partial
# ===================================================================
def build_launch_b(loop_reps=0, pow_plan=None, fp8_dt=False):
    """pow_plan: list over states n=0..15 of ("act",) or ("sq", m) or
    ("mul", m1, m2) describing how dA_n is built (m's are state indices)."""
    if pow_plan is None:
        pow_plan = default_pow_plan()
    nc = bacc.Bacc("TRN2", target_bir_lowering=False, debug=False)
    u_dt_ty = FP8E4 if fp8_dt else BF16
    u_all = nc.dram_tensor("u_all", [KT_U, 128, L], u_dt_ty,
                           kind="ExternalInput")
    u_own = nc.dram_tensor("u_own", [FT, 128, L], BF16, kind="ExternalInput")
    dtw = nc.dram_tensor("dtw", [KT_U, 128, EL], u_dt_ty,
                         kind="ExternalInput")
    dtb = nc.dram_tensor("dtb", [128, FT], F32, kind="ExternalInput")
    acol = nc.dram_tensor("acol", [128, FT * N], F32, kind="ExternalInput")
    bbrd = nc.dram_tensor("bbrd", [N, 128, L], BF16, kind="ExternalInput")
    ccrd = nc.dram_tensor("ccrd", [N, 128, L], BF16, kind="ExternalInput")
    eye = nc.dram_tensor("eye", [128, 128], BF16, kind="ExternalInput")
    dsk = nc.dram_tensor("dsk", [128, FT], F32, kind="ExternalInput")
    ow = nc.dram_tensor("ow", [FT, 128, 8 * 128], BF16, kind="ExternalInput")
    yp = nc.dram_tensor("yp", [D, L], FP16, kind="ExternalOutput")

    with TileContext(nc) as tc:
        if loop_reps:
            tc.race_detector_enabled = False
        with (
            tc.tile_pool(name="big", bufs=1) as big,
            tc.tile_pool(name="uown", bufs=2) as uown,
            tc.tile_pool(name="ustr", bufs=4) as ustr,
            tc.tile_pool(name="bbp", bufs=1) as bbp,
            tc.tile_pool(name="ccp", bufs=1) as ccp,
            tc.tile_pool(name="dpool", bufs=2) as dpool,
            tc.tile_pool(name="powp", bufs=10) as powp,
            tc.tile_pool(name="work", bufs=3) as work,
            tc.tile_pool(name="psd", bufs=1, space="PSUM") as psd,
            tc.tile_pool(name="psy", bufs=1, space="PSUM") as psy,
            tc.tile_pool(name="pso", bufs=2, space="PSUM") as pso,
        ):
            # ---- resident weights/constants ----
            dtw_t = big.tile([128, KT_U * EL], u_dt_ty, tag="dtw",
                             name="dtw")
            for kt in range(KT_U):
                nc.sync.dma_start(dtw_t[:, kt * EL:(kt + 1) * EL],
                                  dtw[kt, :, :])
            ow_t = big.tile([128, FT * 8 * 128], BF16, tag="ow", name="ow")
            for ft in range(FT):
                nc.sync.dma_start(ow_t[:, ft * 1024:(ft + 1) * 1024],
                                  ow[ft, :, :])
            eye_t = big.tile([128, 128], BF16, tag="eye", name="eye")
            nc.sync.dma_start(eye_t[:, :], eye[:, :])
            dtb_t = big.tile([128, FT], F32, tag="dtb", name="dtb")
            nc.sync.dma_start(dtb_t[:, :], dtb[:, :])
            acol_t = big.tile([128, FT * N], F32, tag="acol", name="acol")
            nc.sync.dma_start(acol_t[:, :], acol[:, :])
            dsk_t = big.tile([128, FT], F32, tag="dsk", name="dsk")
            nc.sync.dma_start(dsk_t[:, :], dsk[:, :])
            hlast = big.tile([128, FT * N], F32, tag="hlast", name="hlast")

            def body(_=None, unroll=None):
                for cI in range(NCH):
                    c0 = cI * CW
                    # ---- B/C broadcast tiles for this chunk ----
                    bbt, cct = [], []
                    for n in range(N):
                        bt = bbp.tile([128, CW], BF16, tag=f"bb{n}",
                                      name=f"bb{n}")
                        nc.sync.dma_start(bt[:, :], bbrd[n, :, c0:c0 + CW])
                        bbt.append(bt)
                        ct = ccp.tile([128, CW], BF16, tag=f"cc{n}",
                                      name=f"cc{n}")
                        nc.sync.dma_start(ct[:, :], ccrd[n, :, c0:c0 + CW])
                        cct.append(ct)

                    # ---- stage 1: dt-proj + softplus (per 512 segment) ----
                    delta = [dpool.tile([128, CW], BF16, tag=f"dl{ft}",
                                        name=f"dl{ft}") for ft in range(FT)]
                    uo = [uown.tile([128, CW], BF16, tag=f"uo{ft}",
                                    name=f"uo{ft}") for ft in range(FT)]
                    w_ = [dpool.tile([128, CW], BF16, tag=f"w{ft}",
                                     name=f"w{ft}") for ft in range(FT)]
                    for ft in range(FT):
                        nc.sync.dma_start(uo[ft][:, :],
                                          u_own[ft, :, c0:c0 + CW])
                    psd_scale = (1.0 / (U8_SCALE * DTW8_SCALE)
                                 if fp8_dt else 1.0)
                    for g in range(SEG):
                        t0 = c0 + g * 512
                        psD = [psd.tile([128, 512], F32, tag=f"psD{ft}",
                                        name=f"psD{ft}") for ft in range(FT)]
                        if fp8_dt:
                            for kp in range(KT_U // 2):
                                ut = ustr.tile([128, 2 * 512], FP8E4,
                                               tag="ustr", name="ustr")
                                nc.sync.dma_start(
                                    ut[:, 0:512],
                                    u_all[2 * kp, :, t0:t0 + 512])
                                nc.sync.dma_start(
                                    ut[:, 512:1024],
                                    u_all[2 * kp + 1, :, t0:t0 + 512])
                                rhs3 = ut[:, :].reshape([128, 2, 512])
                                for ft in range(FT):
                                    lhs3 = dtw_t[:, 2 * kp * EL + ft * 128:]
                                    lhs3 = dtw_t[:, :].reshape(
                                        [128, KT_U, EL])[:,
                                        2 * kp:2 * kp + 2,
                                        ft * 128:(ft + 1) * 128]
                                    nc.tensor.matmul(
                                        psD[ft][:, :], lhs3, rhs3,
                                        start=(kp == 0),
                                        stop=(kp == KT_U // 2 - 1),
                                        perf_mode=mybir.MatmulPerfMode.DoubleRow)
                        else:
                            for kt in range(KT_U):
                                ut = ustr.tile([128, 512], BF16, tag="ustr",
                                               name="ustr")
                                nc.sync.dma_start(ut[:, :],
                                                  u_all[kt, :, t0:t0 + 512])
                                for ft in range(FT):
                                    nc.tensor.matmul(
                                        psD[ft][:, :],
                                        dtw_t[:, kt * EL + ft * 128:
                                              kt * EL + (ft + 1) * 128],
                                        ut[:, :], start=(kt == 0),
                                        stop=(kt == KT_U - 1))
                        for ft in range(FT):
                            et = work.tile([128, 512], BF16, tag="et",
                                           name="et")
                            nc.scalar.activation(et[:, :], psD[ft][:, :],
                                                 AF.Exp, scale=psd_scale,
                                                 bias=dtb_t[:, ft:ft + 1])
                            nc.scalar.activation(
                                delta[ft][:, g * 512:(g + 1) * 512],
                                et[:, :], AF.Ln, bias=1.0)
                    for ft in range(FT):
                        nc.vector.scalar_tensor_tensor(
                            w_[ft][:, :], delta[ft][:, :], 1.0, uo[ft][:, :],
                            AL.mult, AL.mult)

                    # ---- stage 2: per-state scans + y ----
                    ysl = []
                    for ft in range(FT):
                        powt = {}
                        psY = [psy.tile([128, 512], F32, tag=f"psY{g}",
                                        name=f"psY{g}") for g in range(SEG)]
                        for n in range(N):
                            plan = pow_plan[n]
                            dA = powp.tile([128, CW], FP16, tag="pow",
                                           name="pow")
                            if plan[0] == "act":
                                nc.scalar.activation(
                                    dA[:, :], delta[ft][:, :], AF.Exp,
                                    scale=acol_t[:, ft * N + n:
                                                 ft * N + n + 1])
                            elif plan[0] == "sq":
                                src = powt[plan[1]]
                                nc.vector.tensor_tensor(
                                    dA[:, :], src[:, :], src[:, :], AL.mult)
                            else:
                                nc.vector.tensor_tensor(
                                    dA[:, :], powt[plan[1]][:, :],
                                    powt[plan[2]][:, :], AL.mult)
                            powt[n] = dA
                            X = work.tile([128, CW], BF16, tag="X", name="X")
                            eng_x = nc.gpsimd if X_ON_GP[n] else nc.vector
                            eng_x.tensor_tensor(X[:, :], w_[ft][:, :],
                                                bbt[n][:, :], AL.mult)
                            h = work.tile([128, CW], BF16, tag="h", name="h")
                            j = ft * N + n
                            init = 0.0 if cI == 0 else hlast[:, j:j + 1]
                            nc.vector.tensor_tensor_scan(
                                h[:, :], dA[:, :], X[:, :], init,
                                AL.mult, AL.add)
                            nc.gpsimd.tensor_copy(hlast[:, j:j + 1],
                                                  h[:, CW - 1:CW])
                            ch = work.tile([128, CW], BF16, tag="ch",
                                           name="ch")
                            eng_c = nc.gpsimd if CH_ON_GP[n] else nc.vector
                            eng_c.tensor_tensor(ch[:, :], h[:, :],
                                                cct[n][:, :], AL.mult)
                            for g in range(SEG):
                                nc.tensor.matmul(
                                    psY[g][:, :], eye_t[:, :],
                                    ch[:, g * 512:(g + 1) * 512],
                                    start=(n == 0), stop=(n == N - 1),
                                    skip_group_check=True)
                        ys = dpool.tile([128, CW], BF16, tag=f"ys{ft}",
                                        name=f"ys{ft}")
                        for g in range(SEG):
                            nc.vector.scalar_tensor_tensor(
                                ys[:, g * 512:(g + 1) * 512],
                                uo[ft][:, g * 512:(g + 1) * 512],
                                dsk_t[:, ft:ft + 1],
                                psY[g][:, :], AL.mult, AL.add)
                        ysl.append(ys)

                    # ---- stage 3: out-proj ----
                    for mt in range(8):
                        for g in range(SEG):
                            psO = pso.tile([128, 512], F32, tag="psO",
                                           name="psO")
                            for ft in range(FT):
                                nc.tensor.matmul(
                                    psO[:, :],
                                    ow_t[:, ft * 1024 + mt * 128:
                                         ft * 1024 + (mt + 1) * 128],
                                    ysl[ft][:, g * 512:(g + 1) * 512],
                                    start=(ft == 0), stop=(ft == FT - 1))
                            ot = work.tile([128, 512], FP16, tag="ot",
                                           name="ot")
                            nc.scalar.activation(ot[:, :], psO[:, :], AF.Copy)
                            nc.sync.dma_start(
                                yp[mt * 128:(mt + 1) * 128,
                                   c0 + g * 512:c0 + (g + 1) * 512],
                                ot[:, :])

            if loop_reps:
                with tc.For_i(0, loop_reps, 1) as _i:
                    body()
            else:
                body()
    nc.compile()
    return nc


def default_pow_plan():
    """Powers p=n+1; evens via squares, odds via Act exp."""
    plan = []
    for n in range(N):
        p = n + 1
        if p % 2 == 0:
            plan.append(("sq", p // 2 - 1))
        else:
            plan.append(("act",))
    return plan


def make_pow_plan(A):
    """Generic: build dA_n as square/product of other states when the A
    columns allow it exactly (within tolerance), else Act exp."""
    plan = []
    cols = [A[:, n] for n in range(N)]
    for n in range(N):
        entry = ("act",)
        for m in range(n):
            if np.allclose(cols[n], 2 * cols[m], rtol=1e-5, atol=1e-7):
                entry = ("sq", m)
                break
        plan.append(entry)
    return plan


# ===================================================================
# Host-side weight preparation
# ===================================================================
def prepare(inputs):
    x = np.asarray(inputs["x"], np.float32)
    conv_w = np.asarray(inputs["conv_w"], np.float32)
    conv_b = np.asarray(inputs["conv_b"], np.float32)
    in_w = np.asarray(inputs["in_w"], np.float32)
    in_b = np.asarray(inputs["in_b"], np.float32)
    A_log = np.asarray(inputs["A_log"], np.float32)
    Dskip = np.asarray(inputs["Dskip"], np.float32)
    dt_w = np.asarray(inputs["dt_w"], np.float32)
    dt_b = np.asarray(inputs["dt_b"], np.float32)
    Bp_w = np.asarray(inputs["Bp_w"], np.float32)
    Bp_b = np.asarray(inputs["Bp_b"], np.float32)
    Cp_w = np.asarray(inputs["Cp_w"], np.float32)
    Cp_b = np.asarray(inputs["Cp_b"], np.float32)
    out_w = np.asarray(inputs["out_w"], np.float32)
    out_b = np.asarray(inputs["out_b"], np.float32)

    # fused conv+in_proj: Wc[f,d,k] = sum_e in_w[f,e] conv_w[e,d,k]
    Wf = (in_w @ conv_w.reshape(E, D * K)).reshape(E, D, K)
    Wf_knl = Wf.transpose(2, 1, 0).reshape(K * D, E)   # [(kap,d), f]
    b_fused = in_w @ conv_b + in_b                      # [E]

    A = -np.exp(A_log)                                  # [E, N]

    prep = {"A": A, "pow_plan": make_pow_plan(A)}
    prep["xT"] = []
    for b in range(B):
        xt = np.zeros((D, L + 2), np.float32)
        xt[:, 1:L + 1] = x[b].T
        prep["xT"].append(_bf16(xt))

    prep["wf"], prep["bfu"], prep["bcw"] = [], [], []
    prep["dtw"], prep["dtb"], prep["acol"] = [], [], []
    prep["dskc"], prep["owk"] = [], []
    for s in range(TPG):
        Fc = slice(s * EL, (s + 1) * EL)
        prep["wf"].append(_bf16(Wf_knl[:, Fc].reshape(KT_X, 128, EL)))
        prep["bfu"].append(
            np.ascontiguousarray(b_fused[Fc].reshape(FT, 128).T,
                                 dtype=np.float32))
        # B/C partial weights: bcw[ft][p, j] with j<16 -> Bp_w, j>=16 -> Cp_w
        bcw = np.empty((FT, 128, 2 * N), np.float32)
        for ft in range(FT):
            rows = slice(s * EL + ft * 128, s * EL + (ft + 1) * 128)
            bcw[ft, :, :N] = Bp_w[:, rows].T
            bcw[ft, :, N:] = Cp_w[:, rows].T
        prep["bcw"].append(_bf16(bcw))
        prep["dtw"].append(_bf16(dt_w[Fc, :].T.reshape(KT_U, 128, EL)))
        prep.setdefault("dtw8", []).append(np.ascontiguousarray(
            (dt_w[Fc, :].T.reshape(KT_U, 128, EL) * DTW8_SCALE
             ).astype(E4M3NP)))
        prep["dtb"].append(
            np.ascontiguousarray(dt_b[Fc].reshape(FT, 128).T,
                                 dtype=np.float32))
        # acol[p, ft*N + n] = A[s*512 + ft*128 + p, n]
        ac = np.empty((128, FT * N), np.float32)
        for ft in range(FT):
            ac[:, ft * N:(ft + 1) * N] = A[s * EL + ft * 128:
                                           s * EL + (ft + 1) * 128, :]
        prep["acol"].append(np.ascontiguousarray(ac))
        prep["dskc"].append(
            np.ascontiguousarray(Dskip[Fc].reshape(FT, 128).T,
                                 dtype=np.float32))
        owk = np.empty((FT, 128, 8 * 128), np.float32)
        for ft in range(FT):
            owk[ft] = out_w[:, s * EL + ft * 128:s * EL + (ft + 1) * 128].T
        prep["owk"].append(_bf16(owk))

    prep["eye"] = _bf16(np.eye(128, dtype=np.float32))
    prep["Bp_b"], prep["Cp_b"] = Bp_b, Cp_b
    prep["out_b"] = out_b
    return prep


# ===================================================================
# Orchestration
# ===================================================================
_CACHE = {}


def _get_kernel_a(loop_reps=0):
    key = ("a", loop_reps, FP8_DT)
    if key not in _CACHE:
        _CACHE[key] = build_launch_a(loop_reps, emit_fp8=FP8_DT)
    return _CACHE[key]


def _get_kernel_b(pow_plan, loop_reps=0):
    key = ("b", loop_reps, FP8_DT)
    if key not in _CACHE:
        _CACHE[key] = build_launch_b(loop_reps, pow_plan, fp8_dt=FP8_DT)
    return _CACHE[key]


def run_launch_a(nca, prep, **kw):
    in_maps = []
    for c in range(NCORES):
        b, s = divmod(c, TPG)
        in_maps.append(dict(xT=prep["xT"][b], wf=prep["wf"][s],
                            bfu=prep["bfu"][s], bcw=prep["bcw"][s]))
    res = bass_utils.run_bass_kernel_spmd(nca, in_maps,
                                          core_ids=list(range(NCORES)), **kw)
    return res.results


def host_exchange(prep, res_a):
    """Gather u, reduce B/C partials, pre-broadcast."""
    u_full, bbrd, ccrd = [], [], []
    u8_full = []
    for b in range(B):
        shards = [np.asarray(res_a[b * TPG + s]["u_out"]) for s in range(TPG)]
        u_full.append(np.concatenate(shards, axis=0))    # [E, L] bf16
        if FP8_DT:
            s8 = [np.asarray(res_a[b * TPG + s]["u8_out"])
                  for s in range(TPG)]
            u8_full.append(np.concatenate(s8, axis=0))
        bc = sum(np.asarray(res_a[b * TPG + s]["bcp"], np.float32)
                 for s in range(TPG))
        Bt = bc[:N] + prep["Bp_b"][:, None]              # [N, L]
        Ct = bc[N:] + prep["Cp_b"][:, None]
        bbrd.append(np.ascontiguousarray(
            np.broadcast_to(Bt.astype(BF16NP)[:, None, :], (N, 128, L))))
        ccrd.append(np.ascontiguousarray(
            np.broadcast_to(Ct.astype(BF16NP)[:, None, :], (N, 128, L))))
    return (u_full, bbrd, ccrd, u8_full)


def make_b_in_maps(prep, u_full, bbrd, ccrd, u8_full=None):
    in_maps = []
    for c in range(NCORES):
        b, s = divmod(c, TPG)
        ub = u_full[b]
        ua = (u8_full[b] if FP8_DT else ub)
        in_maps.append(dict(
            u_all=np.ascontiguousarray(ua.reshape(KT_U, 128, L)),
            u_own=np.ascontiguousarray(
                ub[s * EL:(s + 1) * EL].reshape(FT, 128, L)),
            dtw=(prep["dtw8"][s] if FP8_DT else prep["dtw"][s]),
            dtb=prep["dtb"][s],
            acol=prep["acol"][s], bbrd=bbrd[b], ccrd=ccrd[b],
            eye=prep["eye"], dsk=prep["dskc"][s], ow=prep["owk"][s]))
    return in_maps


def run_launch_b(prep, u_full, bbrd, ccrd, u8_full=None, **kw):
    ncb = _get_kernel_b(prep["pow_plan"], 0)
    in_maps = make_b_in_maps(prep, u_full, bbrd, ccrd, u8_full)
    res = bass_utils.run_bass_kernel_spmd(ncb, in_maps,
                                          core_ids=list(range(NCORES)), **kw)
    return [r["yp"] for r in res.results]


def kernel(**inputs):
    prep = prepare(inputs)
    nca = _get_kernel_a(0)
    res_a = run_launch_a(nca, prep)
    u_full, bbrd, ccrd, u8_full = host_exchange(prep, res_a)
    yps = run_launch_b(prep, u_full, bbrd, ccrd, u8_full)
    out = np.empty((B, L, D), np.float32)
    for b in range(B):
        acc = np.asarray(yps[b * TPG], np.float32)
        for s in range(1, TPG):
            acc = acc + np.asarray(yps[b * TPG + s], np.float32)
        out[b] = acc.T + prep["out_b"][None, :]
    return out
